# revision 3
# baseline (speedup 1.0000x reference)
"""Trainium2 Bass kernel for single-head self-attention (EnhancedSelfAttention).

Reference computation (per batch b):
    q = x @ Wq.T + bq ; k = x @ Wk.T + bk ; v = x @ Wv.T + bv
    out = softmax(q @ k.T / sqrt(D)) @ v

Sharding: 8 cores = 4 batches x 2 query-halves. Each core receives the full
batch slice x[b] (rows rotated so its own 1024 query rows come first), computes
K/V-side quantities for the whole batch, and attention outputs for its half.

Algebraic restructuring used on-device (all matmul operands bf16, fp32 PSUM):
  - softmax over keys is shift-invariant along the key axis, so the bk term
    (constant per query) cancels: bk is never sent to the device.
  - scores^T[sk,sq] = x[sk,:] . r[sq,:] with r = x_q @ C + u, where
    C^T = Wq^T @ Wk (computed on-device from natural-layout weights; no
    weight transposes needed on the q/k path) and u = Wk^T @ bq.
  - v = x @ Wv^T + bv materialized with Wv^T built via PE transposes; bias via
    rank-1 ones x bv matmul into the PSUM accumulation.
  - exp(scores/32) applied by ScalarE straight out of PSUM (no max-shift needed:
    |scores/32| < ~3 for this distribution); softmax denominator via an extra
    N=1 ones-column matmul sharing the attention-weights lhsT; final division
    by per-partition reciprocal on VectorE.
"""

import numpy as np

P = 128
D = 1024
S = 2048
SQ = 1024
ND = D // P     # 8 d-tiles
NE = D // P     # 8 e-tiles
NSK = S // P    # 16 key tiles
FD = 512        # matmul moving free dim
NQC = SQ // FD  # 2 query chunks
SCALE = 1.0 / 32.0

_cached = None


def _build():
    from contextlib import ExitStack

    import concourse.bass as bass
    import concourse.mybir as mybir
    import concourse.tile as tile
    from concourse import bacc
    from concourse.masks import make_identity

    f32 = mybir.dt.float32
    bf16 = mybir.dt.bfloat16
    AF = mybir.ActivationFunctionType

    nc = bacc.Bacc("TRN2", target_bir_lowering=False, debug=False, num_devices=8)

    x_d = nc.declare_dram_parameter("x", [S, D], f32, isOutput=False)
    Wq_d = nc.declare_dram_parameter("Wq", [D, D], f32, isOutput=False)
    Wk_d = nc.declare_dram_parameter("Wk", [D, D], f32, isOutput=False)
    Wv_d = nc.declare_dram_parameter("Wv", [D, D], f32, isOutput=False)
    bq_d = nc.declare_dram_parameter("bq", [D], f32, isOutput=False)
    bv_d = nc.declare_dram_parameter("bv", [D], f32, isOutput=False)
    out_d = nc.declare_dram_parameter("out", [SQ, D], f32, isOutput=True)

    with tile.TileContext(nc) as tc, ExitStack() as ctx:
        const = ctx.enter_context(tc.tile_pool(name="const", bufs=1))
        persist = ctx.enter_context(tc.tile_pool(name="persist", bufs=1))

        # ---- constants ----
        identity = const.tile([P, P], f32)
        make_identity(nc, identity)
        ones_bf = const.tile([P, FD], bf16)
        nc.vector.memset(ones_bf, 1.0)

        bq_stage = const.tile([P, ND], f32)
        nc.sync.dma_start(bq_stage, bq_d.rearrange("(o p) -> p o", p=P))
        bq_col = const.tile([P, ND], bf16)
        nc.any.tensor_copy(out=bq_col, in_=bq_stage)

        bv_stage = const.tile([1, D], f32)
        nc.sync.dma_start(bv_stage, bv_d[None, :])
        bv_row = const.tile([1, D], bf16)
        nc.any.tensor_copy(out=bv_row, in_=bv_stage)

        u_sb = const.tile([P, ND], f32)  # u[d1] = (Wk^T bq)[d1], col per d1-tile

        # ---- persistent bf16 tensors ----
        xT = persist.tile([P, ND, S], bf16)    # x^T  [d, s]
        CT = persist.tile([P, ND, D], bf16)    # C^T  [d2, d1] = Wq^T Wk
        rT = persist.tile([P, ND, SQ], bf16)   # r^T  [d1, sq]
        WvT = persist.tile([P, ND, D], bf16)   # Wv^T [d, e]
        vv = persist.tile([P, NSK, D], bf16)   # v    [sk, e]

        # ---- phase A: natural-layout Wq/Wk -> CT, u ----
        with tc.tile_pool(name="wstage", bufs=3) as wstage, \
             tc.tile_pool(name="wnat", bufs=1) as wnat, \
             tc.tile_pool(name="psumA", bufs=2, space="PSUM") as psumA, \
             tc.tile_pool(name="psumU", bufs=2, space="PSUM") as psumU:
            Wq_nat = wnat.tile([P, NE, D], bf16)
            Wk_nat = wnat.tile([P, NE, D], bf16)
            for W_src, W_bf in ((Wq_d, Wq_nat), (Wk_d, Wk_nat)):
                for t in range(NE):
                    st = wstage.tile([P, D], f32)
                    nc.sync.dma_start(st, W_src[t * P:(t + 1) * P, :])
                    nc.any.tensor_copy(out=W_bf[:, t, :], in_=st)

            # CT[d2, d1] = sum_e Wq[e, d2] * Wk[e, d1]
            for d2t in range(ND):
                for d1c in range(D // FD):
                    ps = psumA.tile([P, FD], f32)
                    for ec in range(NE):
                        nc.tensor.matmul(
                            ps,
                            Wq_nat[:, ec, d2t * P:(d2t + 1) * P],
                            Wk_nat[:, ec, d1c * FD:(d1c + 1) * FD],
                            start=(ec == 0), stop=(ec == NE - 1),
                        )
                    nc.any.tensor_copy(
                        out=CT[:, d2t, d1c * FD:(d1c + 1) * FD], in_=ps)

            # u[d1] = sum_e Wk[e, d1] * bq[e]
            for d1t in range(ND):
                ps = psumU.tile([P, 1], f32)
                for ec in range(NE):
                    nc.tensor.matmul(
                        ps,
                        Wk_nat[:, ec, d1t * P:(d1t + 1) * P],
                        bq_col[:, ec:ec + 1],
                        start=(ec == 0), stop=(ec == NE - 1),
                    )
                nc.any.tensor_copy(out=u_sb[:, d1t:d1t + 1], in_=ps)

        # ---- phase B: x^T and Wv^T via PE transpose ----
        with tc.tile_pool(name="xstage", bufs=3) as xstage, \
             tc.tile_pool(name="psumT", bufs=4, space="PSUM") as psumT:
            for st_i in range(S // P):
                xs = xstage.tile([P, D], f32, tag="xs")
                nc.sync.dma_start(xs, x_d[st_i * P:(st_i + 1) * P, :])
                for dt in range(ND):
                    pt = psumT.tile([P, P], f32)
                    nc.tensor.transpose(pt, xs[:, dt * P:(dt + 1) * P], identity)
                    nc.any.tensor_copy(
                        out=xT[:, dt, st_i * P:(st_i + 1) * P], in_=pt)
            for et in range(NE):
                ws = xstage.tile([P, D], f32, tag="xs")
                nc.sync.dma_start(ws, Wv_d[et * P:(et + 1) * P, :])
                for dt in range(ND):
                    pt = psumT.tile([P, P], f32)
                    nc.tensor.transpose(pt, ws[:, dt * P:(dt + 1) * P], identity)
                    nc.any.tensor_copy(
                        out=WvT[:, dt, et * P:(et + 1) * P], in_=pt)

        # ---- phase C/D: rT and v ----
        with tc.tile_pool(name="psumB", bufs=3, space="PSUM") as psumB:
            # rT[d1, sq] = sum_d2 CT[d2, d1] * xT[d2, sq]  (+ u[d1])
            for d1t in range(ND):
                for qc in range(NQC):
                    ps = psumB.tile([P, FD], f32)
                    for d2c in range(ND):
                        nc.tensor.matmul(
                            ps,
                            CT[:, d2c, d1t * P:(d1t + 1) * P],
                            xT[:, d2c, qc * FD:(qc + 1) * FD],
                            start=(d2c == 0), stop=(d2c == ND - 1),
                        )
                    nc.any.tensor_scalar_add(
                        rT[:, d1t, qc * FD:(qc + 1) * FD], ps,
                        u_sb[:, d1t:d1t + 1])

            # v[sk, e] = sum_d xT[d, sk](as lhsT) * WvT[d, e]  + ones x bv
            for skt in range(NSK):
                for ec2 in range(D // FD):
                    ps = psumB.tile([P, FD], f32)
                    for dc in range(ND):
                        nc.tensor.matmul(
                            ps,
                            xT[:, dc, skt * P:(skt + 1) * P],
                            WvT[:, dc, ec2 * FD:(ec2 + 1) * FD],
                            start=(dc == 0), stop=False,
                        )
                    nc.tensor.matmul(
                        ps,
                        ones_bf[0:1, 0:P],
                        bv_row[0:1, ec2 * FD:(ec2 + 1) * FD],
                        start=False, stop=True,
                    )
                    nc.any.tensor_copy(
                        out=vv[:, skt, ec2 * FD:(ec2 + 1) * FD], in_=ps)

        # ---- phase E: attention ----
        with tc.tile_pool(name="ptpool", bufs=2) as ptpool, \
             tc.tile_pool(name="ostage", bufs=4) as ostage, \
             tc.tile_pool(name="small", bufs=4) as small, \
             tc.tile_pool(name="psumS", bufs=2, space="PSUM") as psumS, \
             tc.tile_pool(name="psumO", bufs=4, space="PSUM") as psumO, \
             tc.tile_pool(name="psumD", bufs=2, space="PSUM") as psumD:
            for qc in range(NQC):
                PT = ptpool.tile([P, NSK, FD], bf16, tag="pt")
                # scores^T[sk, sq] = sum_d xT[d, sk](lhsT) * rT[d, sq]
                for skt in range(NSK):
                    ps = psumS.tile([P, FD], f32)
                    for dc in range(ND):
                        nc.tensor.matmul(
                            ps,
                            xT[:, dc, skt * P:(skt + 1) * P],
                            rT[:, dc, qc * FD:(qc + 1) * FD],
                            start=(dc == 0), stop=(dc == ND - 1),
                        )
                    nc.scalar.activation(PT[:, skt, :], ps, AF.Exp, scale=SCALE)
                # out[sq, e] = sum_sk PT[sk, sq](lhsT) * v[sk, e]; denom via ones col
                for qt in range(FD // P):  # 4 sq-tiles of 128 per chunk
                    po0 = psumO.tile([P, FD], f32, tag="po")
                    po1 = psumO.tile([P, FD], f32, tag="po")
                    pd = psumD.tile([P, 1], f32)
                    for skt in range(NSK):
                        w_lhsT = PT[:, skt, qt * P:(qt + 1) * P]
                        nc.tensor.matmul(po0, w_lhsT, vv[:, skt, 0:FD],
                                         start=(skt == 0), stop=(skt == NSK - 1))
                        nc.tensor.matmul(po1, w_lhsT, vv[:, skt, FD:2 * FD],
                                         start=(skt == 0), stop=(skt == NSK - 1))
                        nc.tensor.matmul(pd, w_lhsT, ones_bf[:, 0:1],
                                         start=(skt == 0), stop=(skt == NSK - 1))
                    rec = small.tile([P, 1], f32)
                    nc.vector.reciprocal(rec, pd)
                    ot0 = ostage.tile([P, FD], f32, tag="ot")
                    ot1 = ostage.tile([P, FD], f32, tag="ot")
                    nc.vector.tensor_scalar_mul(ot0, po0, rec)
                    nc.vector.tensor_scalar_mul(ot1, po1, rec)
                    row0 = (qc * 4 + qt) * P
                    nc.sync.dma_start(out_d[row0:row0 + P, 0:FD], ot0)
                    nc.sync.dma_start(out_d[row0:row0 + P, FD:2 * FD], ot1)

    nc.compile()
    return nc


def _get_nc():
    global _cached
    if _cached is None:
        _cached = _build()
    return _cached


def kernel(x, Wq, bq, Wk, bk, Wv, bv):
    from concourse.bass_utils import run_bass_kernel_spmd

    x = np.ascontiguousarray(np.asarray(x, dtype=np.float32))
    Wq = np.ascontiguousarray(np.asarray(Wq, dtype=np.float32))
    Wk = np.ascontiguousarray(np.asarray(Wk, dtype=np.float32))
    Wv = np.ascontiguousarray(np.asarray(Wv, dtype=np.float32))
    bq = np.ascontiguousarray(np.asarray(bq, dtype=np.float32))
    bv = np.ascontiguousarray(np.asarray(bv, dtype=np.float32))

    nc = _get_nc()
    in_maps = []
    for core in range(8):
        b, h = divmod(core, 2)
        xb = x[b]
        if h:
            xb = np.ascontiguousarray(np.concatenate([xb[SQ:], xb[:SQ]], axis=0))
        in_maps.append(
            {"x": xb, "Wq": Wq, "Wk": Wk, "Wv": Wv, "bq": bq, "bv": bv})

    res = run_bass_kernel_spmd(nc, in_maps, list(range(8)))
    out = np.empty((4, S, D), dtype=np.float32)
    for core in range(8):
        b, h = divmod(core, 2)
        out[b, h * SQ:(h + 1) * SQ, :] = res.results[core]["out"]
    return out


# revision 5
# speedup vs baseline: 1.0757x; 1.0757x over previous
"""Trainium2 Bass kernel for single-head self-attention (EnhancedSelfAttention).

Reference computation (per batch b):
    q = x @ Wq.T + bq ; k = x @ Wk.T + bk ; v = x @ Wv.T + bv
    out = softmax(q @ k.T / sqrt(D)) @ v

Sharding: 8 cores = 4 batches x 2 query-halves. Each core receives the full
batch slice x[b] (rows rotated so its own 1024 query rows come first), computes
K/V-side quantities for the whole batch, and attention outputs for its half.

On-device restructuring (all matmul operands bf16, fp32 PSUM accumulation):
  - softmax over keys is shift-invariant along the key axis, so the bk term
    (constant per query) cancels exactly: bk is never sent to the device.
  - scores^T[sk,sq] = x[sk,:] . r[sq,:] with r = x_q @ C + u, where
    C^T = Wq^T @ Wk is computed on-device from natural-layout weights (no
    weight transposes on the q/k path) and u = Wk^T @ bq.
  - x^T and Wv^T are produced by SWDGE cast-DMA (f32->bf16) into DRAM scratch
    followed by XBAR DMA-transpose loads - zero TensorE/VectorE cost.
  - v = x @ Wv^T + bv materialized with the bias as a rank-1 ones x bv matmul
    inside the PSUM accumulation group.
  - exp(scores/32) applied by ScalarE straight out of PSUM (no max-shift
    needed: |scores|/32 < ~3 for this input distribution); softmax denominator
    via an N=1 ones-column matmul sharing the attention-weights lhsT; final
    division by per-partition reciprocal on VectorE.
"""

import numpy as np

P = 128
D = 1024
S = 2048
SQ = 1024
ND = D // P     # 8 d-tiles
NE = D // P     # 8 e-tiles
NSK = S // P    # 16 key tiles
FD = 512        # matmul moving free dim
NQC = SQ // FD  # 2 query chunks
XCH = 4         # x cast-DMA chunks (along S)
SCALE = 1.0 / 32.0

_cached = None


def _build():
    from contextlib import ExitStack

    import concourse.bass as bass
    import concourse.mybir as mybir
    import concourse.tile as tile
    from concourse import bacc

    f32 = mybir.dt.float32
    bf16 = mybir.dt.bfloat16
    AF = mybir.ActivationFunctionType

    nc = bacc.Bacc("TRN2", target_bir_lowering=False, debug=False, num_devices=8)

    x_d = nc.declare_dram_parameter("x", [S, D], f32, isOutput=False)
    Wq_d = nc.declare_dram_parameter("Wq", [D, D], f32, isOutput=False)
    Wk_d = nc.declare_dram_parameter("Wk", [D, D], f32, isOutput=False)
    Wv_d = nc.declare_dram_parameter("Wv", [D, D], f32, isOutput=False)
    bq_d = nc.declare_dram_parameter("bq", [D], f32, isOutput=False)
    bv_d = nc.declare_dram_parameter("bv", [D], f32, isOutput=False)
    out_d = nc.declare_dram_parameter("out", [SQ, D], f32, isOutput=True)

    with tile.TileContext(nc) as tc, ExitStack() as ctx:
        const = ctx.enter_context(tc.tile_pool(name="const", bufs=1))
        persist = ctx.enter_context(tc.tile_pool(name="persist", bufs=1))
        dram = ctx.enter_context(tc.tile_pool(name="dram", bufs=1, space="DRAM"))

        # ---- bf16 weight loads (SWDGE cast-DMA) ----
        Wq_nat = persist.tile([P, NE, D], bf16)  # Wq[e, d], e on partitions
        Wk_nat = persist.tile([P, NE, D], bf16)
        nc.gpsimd.dma_start(out=Wq_nat, in_=Wq_d.rearrange("(o p) d -> p o d", p=P))
        nc.gpsimd.dma_start(out=Wk_nat, in_=Wk_d.rearrange("(o p) d -> p o d", p=P))

        # ---- x -> bf16 DRAM scratch -> XBAR transpose into SBUF ----
        xT = persist.tile([P, ND, S], bf16)      # x^T [d, s]
        x_bf = dram.tile([S, D], bf16)
        xrows = S // XCH
        for c in range(XCH):
            nc.gpsimd.dma_start(out=x_bf[c * xrows:(c + 1) * xrows, :],
                                in_=x_d[c * xrows:(c + 1) * xrows, :])
            for dt in range(ND):
                nc.sync.dma_start(
                    out=xT[:, dt, c * xrows:(c + 1) * xrows],
                    in_=x_bf[c * xrows:(c + 1) * xrows, dt * P:(dt + 1) * P],
                    transpose=True)

        # ---- Wv -> bf16 DRAM scratch -> XBAR transpose ----
        WvT = persist.tile([P, ND, D], bf16)     # Wv^T [d, e]
        Wv_bf = dram.tile([D, D], bf16)
        nc.gpsimd.dma_start(out=Wv_bf, in_=Wv_d[:, :])
        for dt in range(ND):
            nc.sync.dma_start(out=WvT[:, dt, :],
                              in_=Wv_bf[:, dt * P:(dt + 1) * P], transpose=True)

        # ---- small constants ----
        ones_bf = const.tile([P, FD], bf16)
        nc.vector.memset(ones_bf, 1.0)
        bq_col = const.tile([P, ND], bf16)
        nc.gpsimd.dma_start(out=bq_col, in_=bq_d.rearrange("(o p) -> p o", p=P))
        bv_row = const.tile([1, D], bf16)
        nc.gpsimd.dma_start(out=bv_row, in_=bv_d[None, :])
        u_sb = const.tile([P, ND], f32)  # u[d1] = (Wk^T bq)[d1], col per d1-tile

        CT = persist.tile([P, ND, D], bf16)      # C^T [d2, d1] = Wq^T Wk
        rT = persist.tile([P, ND, SQ], bf16)     # r^T [d1, sq]
        vv = persist.tile([P, NSK, D], bf16)     # v   [sk, e]

        # ---- phase A: CT and u ----
        with tc.tile_pool(name="psumA", bufs=2, space="PSUM") as psumA, \
             tc.tile_pool(name="psumU", bufs=2, space="PSUM") as psumU:
            # CT[d2, d1] = sum_e Wq[e, d2] * Wk[e, d1]
            for d2t in range(ND):
                for d1c in range(D // FD):
                    ps = psumA.tile([P, FD], f32)
                    for ec in range(NE):
                        nc.tensor.matmul(
                            ps,
                            Wq_nat[:, ec, d2t * P:(d2t + 1) * P],
                            Wk_nat[:, ec, d1c * FD:(d1c + 1) * FD],
                            start=(ec == 0), stop=(ec == NE - 1),
                        )
                    nc.any.tensor_copy(
                        out=CT[:, d2t, d1c * FD:(d1c + 1) * FD], in_=ps)
            # u[d1] = sum_e Wk[e, d1] * bq[e]
            for d1t in range(ND):
                ps = psumU.tile([P, 1], f32)
                for ec in range(NE):
                    nc.tensor.matmul(
                        ps,
                        Wk_nat[:, ec, d1t * P:(d1t + 1) * P],
                        bq_col[:, ec:ec + 1],
                        start=(ec == 0), stop=(ec == NE - 1),
                    )
                nc.any.tensor_copy(out=u_sb[:, d1t:d1t + 1], in_=ps)

        # ---- phase C/D: rT and v ----
        with tc.tile_pool(name="psumB", bufs=3, space="PSUM") as psumB:
            # rT[d1, sq] = sum_d2 CT[d2, d1] * xT[d2, sq]  (+ u[d1])
            for d1t in range(ND):
                for qc in range(NQC):
                    ps = psumB.tile([P, FD], f32)
                    for d2c in range(ND):
                        nc.tensor.matmul(
                            ps,
                            CT[:, d2c, d1t * P:(d1t + 1) * P],
                            xT[:, d2c, qc * FD:(qc + 1) * FD],
                            start=(d2c == 0), stop=(d2c == ND - 1),
                        )
                    nc.any.tensor_scalar_add(
                        rT[:, d1t, qc * FD:(qc + 1) * FD], ps,
                        u_sb[:, d1t:d1t + 1])

            # v[sk, e] = sum_d xT[d, sk](as lhsT) * WvT[d, e]  + ones x bv
            for skt in range(NSK):
                for ec2 in range(D // FD):
                    ps = psumB.tile([P, FD], f32)
                    for dc in range(ND):
                        nc.tensor.matmul(
                            ps,
                            xT[:, dc, skt * P:(skt + 1) * P],
                            WvT[:, dc, ec2 * FD:(ec2 + 1) * FD],
                            start=(dc == 0), stop=False,
                        )
                    nc.tensor.matmul(
                        ps,
                        ones_bf[0:1, 0:P],
                        bv_row[0:1, ec2 * FD:(ec2 + 1) * FD],
                        start=False, stop=True,
                    )
                    nc.any.tensor_copy(
                        out=vv[:, skt, ec2 * FD:(ec2 + 1) * FD], in_=ps)

        # ---- phase E: attention ----
        with tc.tile_pool(name="ptpool", bufs=2) as ptpool, \
             tc.tile_pool(name="ostage", bufs=4) as ostage, \
             tc.tile_pool(name="small", bufs=4) as small, \
             tc.tile_pool(name="psumS", bufs=2, space="PSUM") as psumS, \
             tc.tile_pool(name="psumO", bufs=4, space="PSUM") as psumO, \
             tc.tile_pool(name="psumD", bufs=2, space="PSUM") as psumD:
            for qc in range(NQC):
                PT = ptpool.tile([P, NSK, FD], bf16, tag="pt")
                # scores^T[sk, sq] = sum_d xT[d, sk](lhsT) * rT[d, sq]
                for skt in range(NSK):
                    ps = psumS.tile([P, FD], f32)
                    for dc in range(ND):
                        nc.tensor.matmul(
                            ps,
                            xT[:, dc, skt * P:(skt + 1) * P],
                            rT[:, dc, qc * FD:(qc + 1) * FD],
                            start=(dc == 0), stop=(dc == ND - 1),
                        )
                    nc.scalar.activation(PT[:, skt, :], ps, AF.Exp, scale=SCALE)
                # out[sq, e] = sum_sk PT[sk, sq](lhsT) * v[sk, e]; denom via ones
                for qt in range(FD // P):  # 4 sq-tiles of 128 per chunk
                    po0 = psumO.tile([P, FD], f32, tag="po")
                    po1 = psumO.tile([P, FD], f32, tag="po")
                    pd = psumD.tile([P, 1], f32)
                    for skt in range(NSK):
                        w_lhsT = PT[:, skt, qt * P:(qt + 1) * P]
                        nc.tensor.matmul(po0, w_lhsT, vv[:, skt, 0:FD],
                                         start=(skt == 0), stop=(skt == NSK - 1))
                        nc.tensor.matmul(po1, w_lhsT, vv[:, skt, FD:2 * FD],
                                         start=(skt == 0), stop=(skt == NSK - 1))
                        nc.tensor.matmul(pd, w_lhsT, ones_bf[:, 0:1],
                                         start=(skt == 0), stop=(skt == NSK - 1))
                    rec = small.tile([P, 1], f32)
                    nc.vector.reciprocal(rec, pd)
                    ot0 = ostage.tile([P, FD], f32, tag="ot")
                    ot1 = ostage.tile([P, FD], f32, tag="ot")
                    nc.vector.tensor_scalar_mul(ot0, po0, rec)
                    nc.vector.tensor_scalar_mul(ot1, po1, rec)
                    row0 = (qc * 4 + qt) * P
                    nc.sync.dma_start(out_d[row0:row0 + P, 0:FD], ot0)
                    nc.sync.dma_start(out_d[row0:row0 + P, FD:2 * FD], ot1)

    nc.compile()
    return nc


def _get_nc():
    global _cached
    if _cached is None:
        _cached = _build()
    return _cached


def kernel(x, Wq, bq, Wk, bk, Wv, bv):
    from concourse.bass_utils import run_bass_kernel_spmd

    x = np.ascontiguousarray(np.asarray(x, dtype=np.float32))
    Wq = np.ascontiguousarray(np.asarray(Wq, dtype=np.float32))
    Wk = np.ascontiguousarray(np.asarray(Wk, dtype=np.float32))
    Wv = np.ascontiguousarray(np.asarray(Wv, dtype=np.float32))
    bq = np.ascontiguousarray(np.asarray(bq, dtype=np.float32))
    bv = np.ascontiguousarray(np.asarray(bv, dtype=np.float32))

    nc = _get_nc()
    in_maps = []
    for core in range(8):
        b, h = divmod(core, 2)
        xb = x[b]
        if h:
            xb = np.ascontiguousarray(np.concatenate([xb[SQ:], xb[:SQ]], axis=0))
        in_maps.append(
            {"x": xb, "Wq": Wq, "Wk": Wk, "Wv": Wv, "bq": bq, "bv": bv})

    res = run_bass_kernel_spmd(nc, in_maps, list(range(8)))
    out = np.empty((4, S, D), dtype=np.float32)
    for core in range(8):
        b, h = divmod(core, 2)
        out[b, h * SQ:(h + 1) * SQ, :] = res.results[core]["out"]
    return out


# revision 6
# speedup vs baseline: 1.0865x; 1.0100x over previous
"""Trainium2 Bass kernel for single-head self-attention (EnhancedSelfAttention).

Reference computation (per batch b):
    q = x @ Wq.T + bq ; k = x @ Wk.T + bk ; v = x @ Wv.T + bv
    out = softmax(q @ k.T / sqrt(D)) @ v

Sharding: 8 cores = 4 batches x 2 query-halves. Each core receives the full
batch slice x[b] (rows rotated so its own 1024 query rows come first), computes
K/V-side quantities for the whole batch, and attention outputs for its half.

On-device restructuring (all matmul operands bf16, fp32 PSUM accumulation):
  - softmax over keys is shift-invariant along the key axis, so the bk term
    (constant per query) cancels exactly: bk is never sent to the device.
  - scores^T[sk,sq] = x[sk,:] . r[sq,:] with r = x_q @ C + u, where
    C^T = Wq^T @ Wk is computed on-device from natural-layout weights (no
    weight transposes on the q/k path) and u = Wk^T @ bq.
  - x^T and Wv^T are produced by SWDGE cast-DMA (f32->bf16) into DRAM scratch
    followed by XBAR DMA-transpose loads - zero TensorE/VectorE cost.
  - v = x @ Wv^T + bv materialized with the bias as a rank-1 ones x bv matmul
    inside the PSUM accumulation group.
  - exp(scores/32) applied by ScalarE straight out of PSUM (no max-shift
    needed: |scores|/32 < ~3 for this input distribution); softmax denominator
    via an N=1 ones-column matmul sharing the attention-weights lhsT; final
    division by per-partition reciprocal on VectorE.
"""

import numpy as np

P = 128
D = 1024
S = 2048
SQ = 1024
ND = D // P     # 8 d-tiles
NE = D // P     # 8 e-tiles
NSK = S // P    # 16 key tiles
FD = 512        # matmul moving free dim
NQC = SQ // FD  # 2 query chunks
XCH = 4         # x cast-DMA chunks (along S)
SCALE = 1.0 / 32.0

_cached = None


def _build():
    from contextlib import ExitStack

    import concourse.bass as bass
    import concourse.mybir as mybir
    import concourse.tile as tile
    from concourse import bacc

    f32 = mybir.dt.float32
    bf16 = mybir.dt.bfloat16
    AF = mybir.ActivationFunctionType

    nc = bacc.Bacc("TRN2", target_bir_lowering=False, debug=False, num_devices=8)

    x_d = nc.declare_dram_parameter("x", [S, D], f32, isOutput=False)
    Wq_d = nc.declare_dram_parameter("Wq", [D, D], f32, isOutput=False)
    Wk_d = nc.declare_dram_parameter("Wk", [D, D], f32, isOutput=False)
    Wv_d = nc.declare_dram_parameter("Wv", [D, D], f32, isOutput=False)
    bq_d = nc.declare_dram_parameter("bq", [D], f32, isOutput=False)
    bv_d = nc.declare_dram_parameter("bv", [D], f32, isOutput=False)
    out_d = nc.declare_dram_parameter("out", [SQ, D], f32, isOutput=True)

    with tile.TileContext(nc) as tc, ExitStack() as ctx:
        const = ctx.enter_context(tc.tile_pool(name="const", bufs=1))
        persist = ctx.enter_context(tc.tile_pool(name="persist", bufs=1))
        dram = ctx.enter_context(tc.tile_pool(name="dram", bufs=1, space="DRAM"))

        # ---- small constants first: tiny SWDGE DMAs must precede the big
        # casts in the (FIFO) SWDGE queue, or their consumers stall ----
        ones_bf = const.tile([P, FD], bf16)
        nc.vector.memset(ones_bf, 1.0)
        bq_col = const.tile([P, ND], bf16)
        nc.gpsimd.dma_start(out=bq_col, in_=bq_d.rearrange("(o p) -> p o", p=P))
        bv_row = const.tile([1, D], bf16)
        nc.gpsimd.dma_start(out=bv_row, in_=bv_d[None, :])
        u_sb = const.tile([P, ND], f32)  # u[d1] = (Wk^T bq)[d1], col per d1-tile

        # ---- bf16 weight loads (SWDGE cast-DMA), halved for pipelining ----
        Wq_nat = persist.tile([P, NE, D], bf16)  # Wq[e, d], e on partitions
        Wk_nat = persist.tile([P, NE, D], bf16)
        for W_d, W_nat in ((Wq_d, Wq_nat), (Wk_d, Wk_nat)):
            half = NE // 2
            for hh in range(2):
                nc.gpsimd.dma_start(
                    out=W_nat[:, hh * half:(hh + 1) * half, :],
                    in_=W_d[hh * half * P:(hh + 1) * half * P, :].rearrange(
                        "(o p) d -> p o d", p=P))

        # ---- x -> bf16 DRAM scratch -> XBAR transpose into SBUF ----
        # Wv is cast between x chunks 1 and 2, matching consumption order.
        xT = persist.tile([P, ND, S], bf16)      # x^T [d, s]
        WvT = persist.tile([P, ND, D], bf16)     # Wv^T [d, e]
        x_bf = dram.tile([S, D], bf16)
        Wv_bf = dram.tile([D, D], bf16)
        xrows = S // XCH

        def cast_x_chunk(c):
            nc.gpsimd.dma_start(out=x_bf[c * xrows:(c + 1) * xrows, :],
                                in_=x_d[c * xrows:(c + 1) * xrows, :])
            for dt in range(ND):
                nc.sync.dma_start(
                    out=xT[:, dt, c * xrows:(c + 1) * xrows],
                    in_=x_bf[c * xrows:(c + 1) * xrows, dt * P:(dt + 1) * P],
                    transpose=True)

        cast_x_chunk(0)
        cast_x_chunk(1)
        nc.gpsimd.dma_start(out=Wv_bf, in_=Wv_d[:, :])
        for dt in range(ND):
            nc.sync.dma_start(out=WvT[:, dt, :],
                              in_=Wv_bf[:, dt * P:(dt + 1) * P], transpose=True)
        cast_x_chunk(2)
        cast_x_chunk(3)

        CT = persist.tile([P, ND, D], bf16)      # C^T [d2, d1] = Wq^T Wk
        rT = persist.tile([P, ND, SQ], bf16)     # r^T [d1, sq]
        vv = persist.tile([P, NSK, D], bf16)     # v   [sk, e]

        # ---- phase A: CT and u ----
        with tc.tile_pool(name="psumA", bufs=2, space="PSUM") as psumA, \
             tc.tile_pool(name="psumU", bufs=2, space="PSUM") as psumU:
            # CT[d2, d1] = sum_e Wq[e, d2] * Wk[e, d1]
            for d2t in range(ND):
                for d1c in range(D // FD):
                    ps = psumA.tile([P, FD], f32)
                    for ec in range(NE):
                        nc.tensor.matmul(
                            ps,
                            Wq_nat[:, ec, d2t * P:(d2t + 1) * P],
                            Wk_nat[:, ec, d1c * FD:(d1c + 1) * FD],
                            start=(ec == 0), stop=(ec == NE - 1),
                        )
                    nc.any.tensor_copy(
                        out=CT[:, d2t, d1c * FD:(d1c + 1) * FD], in_=ps)
            # u[d1] = sum_e Wk[e, d1] * bq[e]
            for d1t in range(ND):
                ps = psumU.tile([P, 1], f32)
                for ec in range(NE):
                    nc.tensor.matmul(
                        ps,
                        Wk_nat[:, ec, d1t * P:(d1t + 1) * P],
                        bq_col[:, ec:ec + 1],
                        start=(ec == 0), stop=(ec == NE - 1),
                    )
                nc.any.tensor_copy(out=u_sb[:, d1t:d1t + 1], in_=ps)

        # ---- phase C/D: rT and v ----
        with tc.tile_pool(name="psumB", bufs=3, space="PSUM") as psumB:
            # rT[d1, sq] = sum_d2 CT[d2, d1] * xT[d2, sq]  (+ u[d1])
            for d1t in range(ND):
                for qc in range(NQC):
                    ps = psumB.tile([P, FD], f32)
                    for d2c in range(ND):
                        nc.tensor.matmul(
                            ps,
                            CT[:, d2c, d1t * P:(d1t + 1) * P],
                            xT[:, d2c, qc * FD:(qc + 1) * FD],
                            start=(d2c == 0), stop=(d2c == ND - 1),
                        )
                    nc.any.tensor_scalar_add(
                        rT[:, d1t, qc * FD:(qc + 1) * FD], ps,
                        u_sb[:, d1t:d1t + 1])

            # v[sk, e] = sum_d xT[d, sk](as lhsT) * WvT[d, e]  + ones x bv
            for skt in range(NSK):
                for ec2 in range(D // FD):
                    ps = psumB.tile([P, FD], f32)
                    for dc in range(ND):
                        nc.tensor.matmul(
                            ps,
                            xT[:, dc, skt * P:(skt + 1) * P],
                            WvT[:, dc, ec2 * FD:(ec2 + 1) * FD],
                            start=(dc == 0), stop=False,
                        )
                    nc.tensor.matmul(
                        ps,
                        ones_bf[0:1, 0:P],
                        bv_row[0:1, ec2 * FD:(ec2 + 1) * FD],
                        start=False, stop=True,
                    )
                    nc.any.tensor_copy(
                        out=vv[:, skt, ec2 * FD:(ec2 + 1) * FD], in_=ps)

        # ---- phase E: attention ----
        with tc.tile_pool(name="ptpool", bufs=2) as ptpool, \
             tc.tile_pool(name="ostage", bufs=4) as ostage, \
             tc.tile_pool(name="small", bufs=4) as small, \
             tc.tile_pool(name="psumS", bufs=2, space="PSUM") as psumS, \
             tc.tile_pool(name="psumO", bufs=4, space="PSUM") as psumO, \
             tc.tile_pool(name="psumD", bufs=2, space="PSUM") as psumD:
            for qc in range(NQC):
                PT = ptpool.tile([P, NSK, FD], bf16, tag="pt")
                # scores^T[sk, sq] = sum_d xT[d, sk](lhsT) * rT[d, sq]
                for skt in range(NSK):
                    ps = psumS.tile([P, FD], f32)
                    for dc in range(ND):
                        nc.tensor.matmul(
                            ps,
                            xT[:, dc, skt * P:(skt + 1) * P],
                            rT[:, dc, qc * FD:(qc + 1) * FD],
                            start=(dc == 0), stop=(dc == ND - 1),
                        )
                    nc.scalar.activation(PT[:, skt, :], ps, AF.Exp, scale=SCALE)
                # out[sq, e] = sum_sk PT[sk, sq](lhsT) * v[sk, e]; denom via ones
                for qt in range(FD // P):  # 4 sq-tiles of 128 per chunk
                    po0 = psumO.tile([P, FD], f32, tag="po")
                    po1 = psumO.tile([P, FD], f32, tag="po")
                    pd = psumD.tile([P, 1], f32)
                    for skt in range(NSK):
                        w_lhsT = PT[:, skt, qt * P:(qt + 1) * P]
                        nc.tensor.matmul(po0, w_lhsT, vv[:, skt, 0:FD],
                                         start=(skt == 0), stop=(skt == NSK - 1))
                        nc.tensor.matmul(po1, w_lhsT, vv[:, skt, FD:2 * FD],
                                         start=(skt == 0), stop=(skt == NSK - 1))
                        nc.tensor.matmul(pd, w_lhsT, ones_bf[:, 0:1],
                                         start=(skt == 0), stop=(skt == NSK - 1))
                    rec = small.tile([P, 1], f32)
                    nc.vector.reciprocal(rec, pd)
                    ot0 = ostage.tile([P, FD], f32, tag="ot")
                    ot1 = ostage.tile([P, FD], f32, tag="ot")
                    nc.vector.tensor_scalar_mul(ot0, po0, rec)
                    nc.vector.tensor_scalar_mul(ot1, po1, rec)
                    row0 = (qc * 4 + qt) * P
                    nc.sync.dma_start(out_d[row0:row0 + P, 0:FD], ot0)
                    nc.sync.dma_start(out_d[row0:row0 + P, FD:2 * FD], ot1)

    nc.compile()
    return nc


def _get_nc():
    global _cached
    if _cached is None:
        _cached = _build()
    return _cached


def kernel(x, Wq, bq, Wk, bk, Wv, bv):
    from concourse.bass_utils import run_bass_kernel_spmd

    x = np.ascontiguousarray(np.asarray(x, dtype=np.float32))
    Wq = np.ascontiguousarray(np.asarray(Wq, dtype=np.float32))
    Wk = np.ascontiguousarray(np.asarray(Wk, dtype=np.float32))
    Wv = np.ascontiguousarray(np.asarray(Wv, dtype=np.float32))
    bq = np.ascontiguousarray(np.asarray(bq, dtype=np.float32))
    bv = np.ascontiguousarray(np.asarray(bv, dtype=np.float32))

    nc = _get_nc()
    in_maps = []
    for core in range(8):
        b, h = divmod(core, 2)
        xb = x[b]
        if h:
            xb = np.ascontiguousarray(np.concatenate([xb[SQ:], xb[:SQ]], axis=0))
        in_maps.append(
            {"x": xb, "Wq": Wq, "Wk": Wk, "Wv": Wv, "bq": bq, "bv": bv})

    res = run_bass_kernel_spmd(nc, in_maps, list(range(8)))
    out = np.empty((4, S, D), dtype=np.float32)
    for core in range(8):
        b, h = divmod(core, 2)
        out[b, h * SQ:(h + 1) * SQ, :] = res.results[core]["out"]
    return out
